# revision 36
# baseline (speedup 1.0000x reference)
"""GQA attention kernel for Trainium2, data-parallel over batch on 8 NeuronCores.

Per-core problem (2 of 16 batches): X [1024tok, 1024] -> QKV proj -> RoPE ->
causal GQA attention (8 q heads, 4 kv heads, D=128) -> out proj [1024, 1024].

Layout strategy: everything stays in "feature-on-partition" transposed form,
and attention scores are computed TRANSPOSED (ST[tk,tq]) so that exp(ST) is
already the P.T the PV matmul needs -- no transposes of P at all. Matmul
operands are bf16 (fp32 PSUM accumulate).
  XT[hid,tok]   = host-pre-transposed X                   (DRAM -> SBUF)
  V [tok,dkv]   = X @ Wv       (lhsT = XT, rhs = Wv)
  KT[dkv,tok]   = Wk.T @ XT    + RoPE (merged across both local batches)
  QT[dq,tok]    = Wq.T @ XT    + RoPE
  ST_j[tk,tq]   = KT_j.T @ QT[:, 128j:]   per 128-row tk block, causal range
                  + additive -1e30 causal mask on the diagonal block via a
                    second matmul (mask_lhsT.T @ I) into the same PSUM group
  PT_j          = exp(ST_j)               (ACT, psum -> sbuf bf16)
  CS[128,tq]   += ones[128,128].T @ PT_j  (PE, accumulated over j; every
                  row of CS is the softmax colsum -- costs the same N cycles
                  as an M=1 colsum but arrives pre-broadcast across all
                  partitions, so no rank-1 broadcast is needed)
  OT[d,tq]     += V_j.T @ PT_j            (PE, accumulated over j)
  rb[128,tq]    = reciprocal_approx_fast(CS)  (DVE, psum -> sbuf fp32;
                  DVE time scales with free-size per lane, so the [128,T]
                  reciprocal costs the same as a [1,T] one)
  OT_norm       = OT(psum) * rb -> sbuf bf16  (DVE, fused drain)
  Out[tok,hid]  = OT.T @ Wo    (lhsT = OT, rhs = Wo) -> bf16 DMA store

Schedule: the attention inner loop is ACT(exp)-bound while the projections
are PE-bound, so the Q projection is interleaved with attention head units
(attention for head h-2 of BOTH local batches runs between Q-proj chains
for head h); each engine then hides the other's bottleneck. Within a head
unit the S-matmul groups run one j-step ahead of the colsum/PV consumers.
The output projection runs at the end as a dense PE block.

GPSIMD runs ONLY the RoPE sin-muls: mixing gpsimd op types thrashes its
microcode library (~5us per LIBRARY_RELOAD). RoPE scale 1/sqrt(D) is folded
into the Q cos/sin host constants. Output is stored bf16, upcast on host.
"""

import numpy as np
import ml_dtypes
from contextlib import ExitStack

import concourse.bass as bass
import concourse.tile as tile
from concourse import bacc, mybir
from concourse.bass_utils import run_bass_kernel_spmd

B, T, HID = 16, 512, 1024
NH, NKV, D = 8, 4, 128
THETA = 10000.0
NCORES = 8
BL = B // NCORES          # local batches per core
TOK = BL * T              # local tokens
T2 = BL * T               # merged-RoPE width (both local batches)
P = 128
KT_HID = HID // P         # 8 contraction tiles over hidden
NTQ = T // P              # 4 tk/tq tiles per sequence
GROUPS = NH // NKV        # 2 q heads per kv head
FP32 = mybir.dt.float32
BF16 = mybir.dt.bfloat16
NEG_INF = -1e30
BF = ml_dtypes.bfloat16


def _host_consts():
    inv_freq = 1.0 / (THETA ** (np.arange(0, D, 2, dtype=np.float64) / D))
    freqs = np.outer(np.arange(T, dtype=np.float64), inv_freq)    # [T, 64]
    emb = np.concatenate([freqs, freqs], axis=-1)                 # [T, 128]
    cos = np.cos(emb).T                                           # [128, T]
    sin = np.sin(emb).T
    scale = 1.0 / np.sqrt(D)
    # rotate_half sign folded into sin: out = q*cos + qswap*sin_signed where
    # qswap is q with its partition halves swapped
    sin_signed = np.concatenate([-sin[:D // 2], sin[D // 2:]], axis=0)
    # tables doubled along free dim so RoPE runs once per head over both
    # local batches ([P, 2T])
    cos2 = np.tile(cos, (1, BL))
    sin2 = np.tile(sin_signed, (1, BL))
    # additive causal mask for the transposed-S diagonal block, applied on
    # the PE as mask_lhsT.T @ I accumulated into the S psum group.
    # Effective added value at [tk, c] is mask_lhsT[c, tk]: NEG_INF iff
    # c < tk (strictly-lower triangle of ST's diag block = future tokens).
    mask_lhsT = np.zeros((P, P), np.float32)
    iu = np.triu_indices(P, 1)          # c < tk  <=>  upper tri of [c, tk]
    mask_lhsT[iu] = NEG_INF
    return {
        "cos_q2": (cos2 * scale).astype(BF),
        "sin_q2": (sin2 * scale).astype(BF),
        "cos_k2": cos2.astype(BF),
        "sin_k2": sin2.astype(BF),
        "mask_lhsT": mask_lhsT.astype(BF),
        "ident": np.eye(P, dtype=np.float32).astype(BF),
    }


def _rope2(nc, out_sl, raw, cos2_sb, sin2_sb, pool):
    """out = q*cos + rotate_half(q)*sin for q = raw [P, 2T] bf16 in SBUF.

    The partition-half swap of rotate_half runs on the sync-ring DMA
    (compute engines cannot shift partitions between SBUF operands); the
    sign of rotate_half is folded into the host sin constant.
    """
    H = D // 2
    qswap = pool.tile([P, T2], BF16, tag="rope_swap")
    nc.sync.dma_start(out=qswap[0:H], in_=raw[H:P])
    nc.sync.dma_start(out=qswap[H:P], in_=raw[0:H])
    qcos = pool.tile([P, T2], BF16, tag="rope_qcos")
    nc.vector.tensor_mul(qcos, raw, cos2_sb)
    tmp = pool.tile([P, T2], BF16, tag="rope_tmp")
    nc.gpsimd.tensor_mul(tmp, qswap, sin2_sb)
    nc.vector.tensor_add(out_sl, qcos, tmp)


def _build(nc):
    # hidden arrives pre-transposed from the host: [HID, TOK]
    hid_t = nc.dram_tensor("hidden_t", [HID, TOK], BF16,
                           kind="ExternalInput").ap()
    wq = nc.dram_tensor("Wq", [HID, NH * D], BF16, kind="ExternalInput").ap()
    wk = nc.dram_tensor("Wk", [HID, NKV * D], BF16, kind="ExternalInput").ap()
    wv = nc.dram_tensor("Wv", [HID, NKV * D], BF16, kind="ExternalInput").ap()
    wo = nc.dram_tensor("Wo", [NH * D, HID], BF16, kind="ExternalInput").ap()
    cos_q2 = nc.dram_tensor("cos_q2", [P, T2], BF16, kind="ExternalInput").ap()
    sin_q2 = nc.dram_tensor("sin_q2", [P, T2], BF16, kind="ExternalInput").ap()
    cos_k2 = nc.dram_tensor("cos_k2", [P, T2], BF16, kind="ExternalInput").ap()
    sin_k2 = nc.dram_tensor("sin_k2", [P, T2], BF16, kind="ExternalInput").ap()
    mask_d = nc.dram_tensor("mask_lhsT", [P, P], BF16,
                            kind="ExternalInput").ap()
    ident_d = nc.dram_tensor("ident", [P, P], BF16, kind="ExternalInput").ap()
    out = nc.dram_tensor("out", [TOK, HID], BF16, kind="ExternalOutput").ap()

    NTOK_T = TOK // P   # 8 token tiles per core

    with tile.TileContext(nc) as tc, ExitStack() as ctx:
        # ---- pools with cross-phase lifetimes ----
        consts = ctx.enter_context(tc.tile_pool(name="consts", bufs=1))

        ident_sb = consts.tile([P, P], BF16, tag="ident")
        mask_sb = consts.tile([P, P], BF16, tag="maskl")
        cosq_sb = consts.tile([P, T2], BF16, tag="cq")
        sinq_sb = consts.tile([P, T2], BF16, tag="sq")
        cosk_sb = consts.tile([P, T2], BF16, tag="ck")
        sink_sb = consts.tile([P, T2], BF16, tag="sk")
        ones_full = consts.tile([P, P], BF16, tag="ones")
        warm_rhs = consts.tile([P, T], BF16, tag="warm")
        junk = consts.tile([1, 32], FP32, tag="junk")
        # small consts land first so warmup + exp-table preload can start
        nc.sync.dma_start(out=ident_sb, in_=ident_d)
        nc.sync.dma_start(out=mask_sb, in_=mask_d)
        nc.vector.memset(ones_full, 1.0)
        nc.vector.memset(warm_rhs, 0.0)

        qkvpool = ctx.enter_context(tc.tile_pool(name="qkv", bufs=1))
        qt_sb = qkvpool.tile([P, NH, TOK], BF16, tag="qt")     # [d, h, tok]
        kt_sb = qkvpool.tile([P, NKV, TOK], BF16, tag="kt")    # [d, g, tok]
        v_sb = qkvpool.tile([P, NTOK_T, NKV * D], BF16, tag="v")  # [tok,tt,dkv]
        otpool = ctx.enter_context(tc.tile_pool(name="otpool", bufs=1))
        ot_sb = otpool.tile([P, NH, TOK], BF16, tag="ot")      # [d, h, tok]
        wopool = ctx.enter_context(tc.tile_pool(name="wopool", bufs=1))
        wo_sb = wopool.tile([P, KT_HID, HID], BF16, tag="wo")

        with ExitStack() as bigp:
            wpool = bigp.enter_context(tc.tile_pool(name="wpool", bufs=1))
            xtp = bigp.enter_context(tc.tile_pool(name="xtp", bufs=1))
            ropet = bigp.enter_context(tc.tile_pool(name="ropet", bufs=2))

            wq_sb = wpool.tile([P, KT_HID, NH * D], BF16, tag="wq")
            wk_sb = wpool.tile([P, KT_HID, NKV * D], BF16, tag="wk")
            wv_sb = wpool.tile([P, KT_HID, NKV * D], BF16, tag="wv")
            xt_sb = xtp.tile([P, KT_HID, TOK], BF16, tag="xt")  # [hid, k, tok]
            wq_r = wq.rearrange("(k p) n -> p k n", p=P)
            wk_r = wk.rearrange("(k p) n -> p k n", p=P)
            wv_r = wv.rearrange("(k p) n -> p k n", p=P)
            hid_r = hid_t.rearrange("(k p) t -> p k t", p=P)
            # load order = first-use order: X+Wv interleaved (the first
            # V-proj chain needs all of both), Wk, K tables, Wq, Q tables,
            # Wo; alternate the two HWDGE rings per chunk
            engs = [nc.sync, nc.scalar]
            qi = 0

            def ld(out_ap, in_ap):
                nonlocal qi
                engs[qi % 2].dma_start(out=out_ap, in_=in_ap)
                qi += 1

            for k in range(KT_HID):
                ld(xt_sb[:, k, :], hid_r[:, k, :])
                ld(wv_sb[:, k, :], wv_r[:, k, :])
            for k in range(KT_HID):
                ld(wk_sb[:, k, :], wk_r[:, k, :])
            ld(cosk_sb, cos_k2)
            ld(sink_sb, sin_k2)
            for k in range(KT_HID):
                ld(wq_sb[:, k, :], wq_r[:, k, :])
            ld(cosq_sb, cos_q2)
            ld(sinq_sb, sin_q2)
            # Wo on the scalar ring: the sync ring carries the RoPE swaps,
            # which must not queue behind a 2MB transfer
            nc.scalar.dma_start(out=wo_sb,
                                in_=wo.rearrange("(k p) n -> p k n", p=P))

            # ---- phase 1: warmup + V proj + K proj (+ K RoPE) ----
            with ExitStack() as pvk:
                psB = pvk.enter_context(tc.tile_pool(
                    name="psB", bufs=6, space=bass.MemorySpace.PSUM))
                psW = pvk.enter_context(tc.tile_pool(
                    name="psW", bufs=1, space=bass.MemorySpace.PSUM))

                # PE warmup: dependency-free matmuls ahead of the first
                # projection so the HAM clock-gate releases (1.2 -> 2.4GHz)
                # while the input DMAs are in flight. Also preload the ACT
                # exp table set (~2.7us one-time) off the critical path.
                nc.scalar.activation(out=junk, in_=warm_rhs[0:1, 0:32],
                                     func=mybir.ActivationFunctionType.Exp,
                                     bias=0.0, scale=1.0)
                wps = psW.tile([P, T], FP32, tag="warmps")
                for _ in range(22):
                    nc.tensor.matmul(wps, warm_rhs[:, 0:P], warm_rhs,
                                     start=True, stop=True,
                                     skip_group_check=True)

                # V natural: [tok, dkv]. The first chains outpace the HBM
                # load stream; dependency-free filler matmuls between them
                # keep the PE (and its HAM clock) busy during load waits.
                for tt in range(NTOK_T):
                    ps = psB.tile([P, T], FP32, tag="projps")
                    for k in range(KT_HID):
                        nc.tensor.matmul(
                            ps[:, :NKV * D],
                            xt_sb[:, k, tt * P:(tt + 1) * P],
                            wv_sb[:, k, :],
                            start=(k == 0), stop=(k == KT_HID - 1))
                    nc.scalar.copy(v_sb[:, tt, :], ps[:, :NKV * D])
                    if tt < 4:
                        for _ in range(3):
                            nc.tensor.matmul(wps, warm_rhs[:, 0:P], warm_rhs,
                                             start=True, stop=True,
                                             skip_group_check=True)
                # KT = Wk.T @ XT + RoPE (merged across both local batches)
                for g in range(NKV):
                    raw = ropet.tile([P, T2], BF16, tag="rope_raw")
                    for c in range(BL):
                        ps = psB.tile([P, T], FP32, tag="projps")
                        for k in range(KT_HID):
                            nc.tensor.matmul(
                                ps,
                                wk_sb[:, k, g * P:(g + 1) * P],
                                xt_sb[:, k, c * T:(c + 1) * T],
                                start=(k == 0), stop=(k == KT_HID - 1))
                        nc.scalar.copy(raw[:, c * T:(c + 1) * T], ps)
                    _rope2(nc, kt_sb[:, g, :], raw, cosk_sb, sink_sb, ropet)

            # ---- phase 2: Q proj interleaved with attention ----
            with ExitStack() as pqa:
                ptpool = pqa.enter_context(tc.tile_pool(name="ptpool",
                                                        bufs=2))
                normp = pqa.enter_context(tc.tile_pool(name="normp", bufs=2))
                opool = pqa.enter_context(tc.tile_pool(name="opool", bufs=2))
                psS = pqa.enter_context(tc.tile_pool(
                    name="psS", bufs=2, space=bass.MemorySpace.PSUM))
                psO = pqa.enter_context(tc.tile_pool(
                    name="psO", bufs=2, space=bass.MemorySpace.PSUM))
                psC = pqa.enter_context(tc.tile_pool(
                    name="psC", bufs=2, space=bass.MemorySpace.PSUM))

                def emit_s_group(b, h, j):
                    g = h // GROUPS
                    lo = j * P
                    st_ps = psS.tile([P, T], FP32, tag="sps")
                    nc.tensor.matmul(
                        st_ps[:, lo:T],
                        kt_sb[:, g, b * T + lo: b * T + lo + P],
                        qt_sb[:, h, b * T + lo: (b + 1) * T],
                        start=True, stop=False, skip_group_check=True)
                    # additive -1e30 causal mask on the diagonal block, on
                    # the PE (frees GPSIMD + shortens the exp chain)
                    nc.tensor.matmul(
                        st_ps[:, lo:lo + P], mask_sb, ident_sb,
                        start=False, stop=True, skip_group_check=True)
                    return st_ps

                def attn_head(b, h):
                    g = h // GROUPS
                    o_ps = psO.tile([P, T], FP32, tag="ops")
                    cs_ps = psC.tile([P, T], FP32, tag="cps")
                    st_next = emit_s_group(b, h, 0)
                    for j in range(NTQ):
                        lo = j * P
                        st_ps = st_next
                        if j + 1 < NTQ:
                            st_next = emit_s_group(b, h, j + 1)
                        # exp -> PT_j, already transposed for the PV matmul
                        # (no row-max: logits are O(1) by construction)
                        pt_t = ptpool.tile([P, T], BF16, tag=f"pt{j}")
                        nc.scalar.activation(
                            out=pt_t[:, lo:T], in_=st_ps[:, lo:T],
                            func=mybir.ActivationFunctionType.Exp,
                            bias=0.0, scale=1.0)
                        # colsum += ones.T @ PT_j ; OT += V_j.T @ PT_j
                        nc.tensor.matmul(
                            cs_ps[:, lo:T] if j else cs_ps[:, :],
                            ones_full,
                            pt_t[:, lo:T],
                            start=(j == 0), stop=(j == NTQ - 1),
                            skip_group_check=True)
                        nc.tensor.matmul(
                            o_ps[:, lo:T] if j else o_ps[:, :],
                            v_sb[:, b * NTQ + j, g * D:(g + 1) * D],
                            pt_t[:, lo:T],
                            start=(j == 0), stop=(j == NTQ - 1),
                            skip_group_check=True)
                    # per-head normalization: the colsum arrives already
                    # broadcast across partitions, so this is just recip
                    # (DVE, psum->sbuf) + fused drain-multiply (DVE)
                    rb = normp.tile([P, T], FP32, tag="rb")
                    nc.vector.reciprocal_approx_fast(rb, cs_ps)
                    nc.vector.tensor_mul(
                        ot_sb[:, h, b * T:(b + 1) * T], o_ps, rb)

                # attention for head h-2 (both batches) runs between Q-proj
                # chains for head h: the RoPE chain of head h has ~2 slots
                # of latency budget, and ACT's exp stream hides under the
                # PE-dense Q chains. psB2 (the Q-proj psum double-buffer)
                # lives in a nested scope so its 2 banks hand over to psD
                # for the flush/output-projection interleave.
                with ExitStack() as pqb:
                    psB2 = pqb.enter_context(tc.tile_pool(
                        name="psB2", bufs=2, space=bass.MemorySpace.PSUM))

                    def qproj(h):
                        raw = ropet.tile([P, T2], BF16, tag="rope_raw")
                        for c in range(BL):
                            ps = psB2.tile([P, T], FP32, tag="projps")
                            for k in range(KT_HID):
                                nc.tensor.matmul(
                                    ps,
                                    wq_sb[:, k, h * P:(h + 1) * P],
                                    xt_sb[:, k, c * T:(c + 1) * T],
                                    start=(k == 0), stop=(k == KT_HID - 1))
                            nc.scalar.copy(raw[:, c * T:(c + 1) * T], ps)
                        _rope2(nc, qt_sb[:, h, :], raw, cosq_sb, sinq_sb,
                               ropet)

                    qproj(0)
                    for h in range(1, NH):
                        qproj(h)
                        attn_head(0, h - 1)
                        attn_head(1, h - 1)

                psD = pqa.enter_context(tc.tile_pool(
                    name="psD", bufs=2, space=bass.MemorySpace.PSUM))
                NCH = HID // T  # 2 chunks of 512

                def phase_d_tile(tt):
                    o_tile = opool.tile([P, HID], BF16, tag="o")
                    for cchunk in range(NCH):
                        ps = psD.tile([P, T], FP32, tag="dps")
                        for k in range(KT_HID):
                            nc.tensor.matmul(
                                ps,
                                ot_sb[:, k, tt * P:(tt + 1) * P],
                                wo_sb[:, k, cchunk * T:(cchunk + 1) * T],
                                start=(k == 0), stop=(k == KT_HID - 1))
                        # alternate engines so the copies run in parallel
                        if cchunk == 0:
                            nc.vector.tensor_copy(o_tile[:, 0:T], ps)
                        else:
                            nc.scalar.copy(o_tile[:, T:HID], ps)
                    nc.sync.dma_start(
                        out=out[tt * P:(tt + 1) * P, :], in_=o_tile)

                # flush the last head; batch 0 finishes first so its
                # PE-dense output projection interleaves with batch 1's
                # ACT-bound final attention head
                attn_head(0, NH - 1)
                phase_d_tile(0)
                attn_head(1, NH - 1)
                phase_d_tile(1)
                for tt in range(2, NTOK_T):
                    phase_d_tile(tt)


_COMPILED = None


def _get_compiled():
    global _COMPILED
    if _COMPILED is None:
        nc = bacc.Bacc("TRN2", target_bir_lowering=False, debug=False)
        _build(nc)
        nc.compile()
        _COMPILED = nc
    return _COMPILED


def kernel(hidden_states, Wq, Wk, Wv, Wo, _trace=False, _trace_kwargs=None):
    hs = np.asarray(hidden_states, dtype=np.float32).astype(BF)
    wq = np.ascontiguousarray(np.asarray(Wq, dtype=np.float32).astype(BF))
    wk = np.ascontiguousarray(np.asarray(Wk, dtype=np.float32).astype(BF))
    wv = np.ascontiguousarray(np.asarray(Wv, dtype=np.float32).astype(BF))
    wo = np.ascontiguousarray(np.asarray(Wo, dtype=np.float32).astype(BF))
    consts = _host_consts()
    nc = _get_compiled()
    in_maps = []
    for c in range(NCORES):
        # ship X pre-transposed ([HID, TOK]) so the kernel's lhs/rhs layouts
        # need no on-chip transpose of X at all
        shard_t = np.ascontiguousarray(
            hs[BL * c: BL * (c + 1)].reshape(TOK, HID).T)
        in_maps.append({"hidden_t": shard_t, "Wq": wq, "Wk": wk, "Wv": wv,
                        "Wo": wo, **consts})
    res = run_bass_kernel_spmd(
        nc, in_maps, list(range(NCORES)), trace=_trace,
        **(_trace_kwargs or {}))
    outs = [r["out"].astype(np.float32).reshape(BL, T, HID)
            for r in res.results]
    full = np.concatenate(outs, axis=0)
    if _trace:
        return full, res
    return full


# revision 38
# speedup vs baseline: 1.1649x; 1.1649x over previous
"""GQA attention kernel for Trainium2, data-parallel over batch on 8 NeuronCores.

Per-core problem (2 of 16 batches): X [1024tok, 1024] -> QKV proj -> RoPE ->
causal GQA attention (8 q heads, 4 kv heads, D=128) -> out proj [1024, 1024].

Layout strategy: everything stays in "feature-on-partition" transposed form,
and attention scores are computed TRANSPOSED (ST[tk,tq]) so that exp(ST) is
already the P.T the PV matmul needs -- no transposes of P at all. Matmul
operands are bf16 (fp32 PSUM accumulate).
  XT[hid,tok]   = host-pre-transposed X                   (DRAM -> SBUF)
  V [tok,dkv]   = X @ Wv       (lhsT = XT, rhs = Wv)
  KT[dkv,tok]   = Wk.T @ XT    + RoPE (merged across both local batches)
  QT[dq,tok]    = Wq.T @ XT    + RoPE
  ST_j[tk,tq]   = KT_j.T @ QT[:, 128j:]   per 128-row tk block, causal range
                  + additive -1e30 causal mask on the diagonal block via a
                    second matmul (mask_lhsT.T @ I) into the same PSUM group
  PT_j          = exp(ST_j)               (ACT, psum -> sbuf bf16)
  CS[128,tq]   += ones[128,128].T @ PT_j  (PE, accumulated over j; every
                  row of CS is the softmax colsum -- costs the same N cycles
                  as an M=1 colsum but arrives pre-broadcast across all
                  partitions, so no rank-1 broadcast is needed)
  OT[d,tq]     += V_j.T @ PT_j            (PE, accumulated over j)
  rb[128,tq]    = reciprocal_approx_fast(CS)  (DVE, psum -> sbuf fp32;
                  DVE time scales with free-size per lane, so the [128,T]
                  reciprocal costs the same as a [1,T] one)
  OT_norm       = OT(psum) * rb -> sbuf bf16  (DVE, fused drain)
  Out[tok,hid]  = OT.T @ Wo    (lhsT = OT, rhs = Wo) -> bf16 DMA store

Schedule: the attention inner loop is ACT(exp)-bound while the projections
are PE-bound, so the Q projection is interleaved with attention head units
(attention for head h-2 of BOTH local batches runs between Q-proj chains
for head h); each engine then hides the other's bottleneck. Within a head
unit the S-matmul groups run one j-step ahead of the colsum/PV consumers.
The output projection runs at the end as a dense PE block.

GPSIMD runs ONLY the RoPE sin-muls: mixing gpsimd op types thrashes its
microcode library (~5us per LIBRARY_RELOAD). RoPE scale 1/sqrt(D) is folded
into the Q cos/sin host constants. Output is stored bf16, upcast on host.
"""

import numpy as np
import ml_dtypes
from contextlib import ExitStack

import concourse.bass as bass
import concourse.tile as tile
from concourse import bacc, mybir
from concourse.bass_utils import run_bass_kernel_spmd

B, T, HID = 16, 512, 1024
NH, NKV, D = 8, 4, 128
THETA = 10000.0
NCORES = 8
BL = B // NCORES          # local batches per core
TOK = BL * T              # local tokens
T2 = BL * T               # merged-RoPE width (both local batches)
P = 128
KT_HID = HID // P         # 8 contraction tiles over hidden
NTQ = T // P              # 4 tk/tq tiles per sequence
GROUPS = NH // NKV        # 2 q heads per kv head
FP32 = mybir.dt.float32
BF16 = mybir.dt.bfloat16
NEG_INF = -1e30
BF = ml_dtypes.bfloat16


def _host_consts():
    inv_freq = 1.0 / (THETA ** (np.arange(0, D, 2, dtype=np.float64) / D))
    freqs = np.outer(np.arange(T, dtype=np.float64), inv_freq)    # [T, 64]
    emb = np.concatenate([freqs, freqs], axis=-1)                 # [T, 128]
    cos = np.cos(emb).T                                           # [128, T]
    sin = np.sin(emb).T
    scale = 1.0 / np.sqrt(D)
    # rotate_half sign folded into sin: out = q*cos + qswap*sin_signed where
    # qswap is q with its partition halves swapped
    sin_signed = np.concatenate([-sin[:D // 2], sin[D // 2:]], axis=0)
    # tables doubled along free dim so RoPE runs once per head over both
    # local batches ([P, 2T])
    cos2 = np.tile(cos, (1, BL))
    sin2 = np.tile(sin_signed, (1, BL))
    # additive causal mask for the transposed-S diagonal block, applied on
    # the PE as mask_lhsT.T @ I accumulated into the S psum group.
    # Effective added value at [tk, c] is mask_lhsT[c, tk]: NEG_INF iff
    # c < tk (strictly-lower triangle of ST's diag block = future tokens).
    mask_lhsT = np.zeros((P, P), np.float32)
    iu = np.triu_indices(P, 1)          # c < tk  <=>  upper tri of [c, tk]
    mask_lhsT[iu] = NEG_INF
    return {
        "cos_q2": (cos2 * scale).astype(BF),
        "sin_q2": (sin2 * scale).astype(BF),
        "cos_k2": cos2.astype(BF),
        "sin_k2": sin2.astype(BF),
        "mask_lhsT": mask_lhsT.astype(BF),
        "ident": np.eye(P, dtype=np.float32).astype(BF),
    }


def _rope2(nc, out_sl, raw, cos2_sb, sin2_sb, pool):
    """out = q*cos + rotate_half(q)*sin for q = raw [P, 2T] bf16 in SBUF.

    The partition-half swap of rotate_half runs on the sync-ring DMA
    (compute engines cannot shift partitions between SBUF operands); the
    sign of rotate_half is folded into the host sin constant.
    """
    H = D // 2
    qswap = pool.tile([P, T2], BF16, tag="rope_swap")
    nc.sync.dma_start(out=qswap[0:H], in_=raw[H:P])
    nc.sync.dma_start(out=qswap[H:P], in_=raw[0:H])
    qcos = pool.tile([P, T2], BF16, tag="rope_qcos")
    nc.vector.tensor_mul(qcos, raw, cos2_sb)
    tmp = pool.tile([P, T2], BF16, tag="rope_tmp")
    nc.gpsimd.tensor_mul(tmp, qswap, sin2_sb)
    nc.vector.tensor_add(out_sl, qcos, tmp)


def _build(nc):
    # hidden arrives pre-transposed from the host: [HID, TOK]
    hid_t = nc.dram_tensor("hidden_t", [HID, TOK], BF16,
                           kind="ExternalInput").ap()
    wq = nc.dram_tensor("Wq", [HID, NH * D], BF16, kind="ExternalInput").ap()
    wk = nc.dram_tensor("Wk", [HID, NKV * D], BF16, kind="ExternalInput").ap()
    wv = nc.dram_tensor("Wv", [HID, NKV * D], BF16, kind="ExternalInput").ap()
    wo = nc.dram_tensor("Wo", [NH * D, HID], BF16, kind="ExternalInput").ap()
    cos_q2 = nc.dram_tensor("cos_q2", [P, T2], BF16, kind="ExternalInput").ap()
    sin_q2 = nc.dram_tensor("sin_q2", [P, T2], BF16, kind="ExternalInput").ap()
    cos_k2 = nc.dram_tensor("cos_k2", [P, T2], BF16, kind="ExternalInput").ap()
    sin_k2 = nc.dram_tensor("sin_k2", [P, T2], BF16, kind="ExternalInput").ap()
    mask_d = nc.dram_tensor("mask_lhsT", [P, P], BF16,
                            kind="ExternalInput").ap()
    ident_d = nc.dram_tensor("ident", [P, P], BF16, kind="ExternalInput").ap()
    out = nc.dram_tensor("out", [TOK, HID], BF16, kind="ExternalOutput").ap()

    NTOK_T = TOK // P   # 8 token tiles per core

    with tile.TileContext(nc) as tc, ExitStack() as ctx:
        # ---- pools with cross-phase lifetimes ----
        consts = ctx.enter_context(tc.tile_pool(name="consts", bufs=1))

        ident_sb = consts.tile([P, P], BF16, tag="ident")
        mask_sb = consts.tile([P, P], BF16, tag="maskl")
        cosq_sb = consts.tile([P, T2], BF16, tag="cq")
        sinq_sb = consts.tile([P, T2], BF16, tag="sq")
        cosk_sb = consts.tile([P, T2], BF16, tag="ck")
        sink_sb = consts.tile([P, T2], BF16, tag="sk")
        ones_full = consts.tile([P, P], BF16, tag="ones")
        warm_rhs = consts.tile([P, T], BF16, tag="warm")
        junk = consts.tile([1, 32], FP32, tag="junk")
        # small consts land first so warmup + exp-table preload can start
        nc.sync.dma_start(out=ident_sb, in_=ident_d)
        nc.sync.dma_start(out=mask_sb, in_=mask_d)
        nc.vector.memset(ones_full, 1.0)
        nc.vector.memset(warm_rhs, 0.0)

        qkvpool = ctx.enter_context(tc.tile_pool(name="qkv", bufs=1))
        qt_sb = qkvpool.tile([P, NH, TOK], BF16, tag="qt")     # [d, h, tok]
        kt_sb = qkvpool.tile([P, NKV, TOK], BF16, tag="kt")    # [d, g, tok]
        v_sb = qkvpool.tile([P, NTOK_T, NKV * D], BF16, tag="v")  # [tok,tt,dkv]
        otpool = ctx.enter_context(tc.tile_pool(name="otpool", bufs=1))
        ot_sb = otpool.tile([P, NH, TOK], BF16, tag="ot")      # [d, h, tok]
        wopool = ctx.enter_context(tc.tile_pool(name="wopool", bufs=1))
        wo_sb = wopool.tile([P, KT_HID, HID], BF16, tag="wo")

        with ExitStack() as bigp:
            wpool = bigp.enter_context(tc.tile_pool(name="wpool", bufs=1))
            xtp = bigp.enter_context(tc.tile_pool(name="xtp", bufs=1))
            ropet = bigp.enter_context(tc.tile_pool(name="ropet", bufs=2))

            wq_sb = wpool.tile([P, KT_HID, NH * D], BF16, tag="wq")
            wk_sb = wpool.tile([P, KT_HID, NKV * D], BF16, tag="wk")
            wv_sb = wpool.tile([P, KT_HID, NKV * D], BF16, tag="wv")
            xt_sb = xtp.tile([P, KT_HID, TOK], BF16, tag="xt")  # [hid, k, tok]
            wq_r = wq.rearrange("(k p) n -> p k n", p=P)
            wk_r = wk.rearrange("(k p) n -> p k n", p=P)
            wv_r = wv.rearrange("(k p) n -> p k n", p=P)
            hid_r = hid_t.rearrange("(k p) t -> p k t", p=P)
            # load order = first-use order: X+Wv interleaved (the first
            # V-proj chain needs all of both), Wk, K tables, Wq, Q tables,
            # Wo; alternate the two HWDGE rings per chunk
            engs = [nc.sync, nc.scalar]
            qi = 0

            def ld(out_ap, in_ap):
                nonlocal qi
                engs[qi % 2].dma_start(out=out_ap, in_=in_ap)
                qi += 1

            for k in range(KT_HID):
                ld(xt_sb[:, k, :], hid_r[:, k, :])
                ld(wv_sb[:, k, :], wv_r[:, k, :])
            for k in range(KT_HID):
                ld(wk_sb[:, k, :], wk_r[:, k, :])
            ld(cosk_sb, cos_k2)
            ld(sink_sb, sin_k2)
            for k in range(KT_HID):
                ld(wq_sb[:, k, :], wq_r[:, k, :])
            ld(cosq_sb, cos_q2)
            ld(sinq_sb, sin_q2)
            # Wo on the scalar ring: the sync ring carries the RoPE swaps,
            # which must not queue behind a 2MB transfer
            nc.scalar.dma_start(out=wo_sb,
                                in_=wo.rearrange("(k p) n -> p k n", p=P))

            # ---- phase 1: warmup + V proj + K proj (+ K RoPE) ----
            with ExitStack() as pvk:
                psB = pvk.enter_context(tc.tile_pool(
                    name="psB", bufs=6, space=bass.MemorySpace.PSUM))
                psW = pvk.enter_context(tc.tile_pool(
                    name="psW", bufs=1, space=bass.MemorySpace.PSUM))

                # PE warmup: dependency-free matmuls ahead of the first
                # projection so the HAM clock-gate releases (1.2 -> 2.4GHz)
                # while the input DMAs are in flight. Also preload the ACT
                # exp table set (~2.7us one-time) off the critical path.
                nc.scalar.activation(out=junk, in_=warm_rhs[0:1, 0:32],
                                     func=mybir.ActivationFunctionType.Exp,
                                     bias=0.0, scale=1.0)
                wps = psW.tile([P, T], FP32, tag="warmps")
                for _ in range(22):
                    nc.tensor.matmul(wps, warm_rhs[:, 0:P], warm_rhs,
                                     start=True, stop=True,
                                     skip_group_check=True)

                # V natural: [tok, dkv]. The first chains outpace the HBM
                # load stream; dependency-free filler matmuls between them
                # keep the PE (and its HAM clock) busy during load waits.
                for tt in range(NTOK_T):
                    ps = psB.tile([P, T], FP32, tag="projps")
                    for k in range(KT_HID):
                        nc.tensor.matmul(
                            ps[:, :NKV * D],
                            xt_sb[:, k, tt * P:(tt + 1) * P],
                            wv_sb[:, k, :],
                            start=(k == 0), stop=(k == KT_HID - 1))
                    nc.scalar.copy(v_sb[:, tt, :], ps[:, :NKV * D])
                    if tt < 4:
                        for _ in range(3):
                            nc.tensor.matmul(wps, warm_rhs[:, 0:P], warm_rhs,
                                             start=True, stop=True,
                                             skip_group_check=True)
                # KT = Wk.T @ XT + RoPE (merged across both local batches)
                for g in range(NKV):
                    raw = ropet.tile([P, T2], BF16, tag="rope_raw")
                    for c in range(BL):
                        ps = psB.tile([P, T], FP32, tag="projps")
                        for k in range(KT_HID):
                            nc.tensor.matmul(
                                ps,
                                wk_sb[:, k, g * P:(g + 1) * P],
                                xt_sb[:, k, c * T:(c + 1) * T],
                                start=(k == 0), stop=(k == KT_HID - 1))
                        nc.scalar.copy(raw[:, c * T:(c + 1) * T], ps)
                    _rope2(nc, kt_sb[:, g, :], raw, cosk_sb, sink_sb, ropet)

            # ---- phase 2: Q proj interleaved with attention ----
            with ExitStack() as pqa:
                ptpool = pqa.enter_context(tc.tile_pool(name="ptpool",
                                                        bufs=2))
                normp = pqa.enter_context(tc.tile_pool(name="normp", bufs=2))
                opool = pqa.enter_context(tc.tile_pool(name="opool", bufs=2))
                psS = pqa.enter_context(tc.tile_pool(
                    name="psS", bufs=2, space=bass.MemorySpace.PSUM))
                psO = pqa.enter_context(tc.tile_pool(
                    name="psO", bufs=2, space=bass.MemorySpace.PSUM))
                psC = pqa.enter_context(tc.tile_pool(
                    name="psC", bufs=2, space=bass.MemorySpace.PSUM))

                def emit_s_group(b, h, j):
                    g = h // GROUPS
                    lo = j * P
                    st_ps = psS.tile([P, T], FP32, tag="sps")
                    nc.tensor.matmul(
                        st_ps[:, lo:T],
                        kt_sb[:, g, b * T + lo: b * T + lo + P],
                        qt_sb[:, h, b * T + lo: (b + 1) * T],
                        start=True, stop=False, skip_group_check=True)
                    # additive -1e30 causal mask on the diagonal block, on
                    # the PE (frees GPSIMD + shortens the exp chain)
                    nc.tensor.matmul(
                        st_ps[:, lo:lo + P], mask_sb, ident_sb,
                        start=False, stop=True, skip_group_check=True)
                    return st_ps

                def attn_head(b, h):
                    g = h // GROUPS
                    o_ps = psO.tile([P, T], FP32, tag="ops")
                    cs_ps = psC.tile([P, T], FP32, tag="cps")
                    st_next = emit_s_group(b, h, 0)
                    for j in range(NTQ):
                        lo = j * P
                        st_ps = st_next
                        if j + 1 < NTQ:
                            st_next = emit_s_group(b, h, j + 1)
                        # exp -> PT_j, already transposed for the PV matmul
                        # (no row-max: logits are O(1) by construction)
                        pt_t = ptpool.tile([P, T], BF16, tag=f"pt{j}")
                        nc.scalar.activation(
                            out=pt_t[:, lo:T], in_=st_ps[:, lo:T],
                            func=mybir.ActivationFunctionType.Exp,
                            bias=0.0, scale=1.0)
                        # colsum += ones.T @ PT_j ; OT += V_j.T @ PT_j
                        nc.tensor.matmul(
                            cs_ps[:, lo:T] if j else cs_ps[:, :],
                            ones_full,
                            pt_t[:, lo:T],
                            start=(j == 0), stop=(j == NTQ - 1),
                            skip_group_check=True)
                        nc.tensor.matmul(
                            o_ps[:, lo:T] if j else o_ps[:, :],
                            v_sb[:, b * NTQ + j, g * D:(g + 1) * D],
                            pt_t[:, lo:T],
                            start=(j == 0), stop=(j == NTQ - 1),
                            skip_group_check=True)
                    # per-head normalization: the colsum arrives already
                    # broadcast across partitions, so this is just recip
                    # (DVE, psum->sbuf) + fused drain-multiply (DVE)
                    rb = normp.tile([P, T], FP32, tag="rb")
                    nc.vector.reciprocal_approx_fast(rb, cs_ps)
                    nc.vector.tensor_mul(
                        ot_sb[:, h, b * T:(b + 1) * T], o_ps, rb)

                # attention for head h-2 (both batches) runs between Q-proj
                # chains for head h: the RoPE chain of head h has ~2 slots
                # of latency budget, and ACT's exp stream hides under the
                # PE-dense Q chains. psB2 (the Q-proj psum double-buffer)
                # lives in a nested scope so its 2 banks hand over to psD
                # for the flush/output-projection interleave.
                with ExitStack() as pqb:
                    psB2 = pqb.enter_context(tc.tile_pool(
                        name="psB2", bufs=2, space=bass.MemorySpace.PSUM))

                    def qproj(h):
                        raw = ropet.tile([P, T2], BF16, tag="rope_raw")
                        for c in range(BL):
                            ps = psB2.tile([P, T], FP32, tag="projps")
                            for k in range(KT_HID):
                                nc.tensor.matmul(
                                    ps,
                                    wq_sb[:, k, h * P:(h + 1) * P],
                                    xt_sb[:, k, c * T:(c + 1) * T],
                                    start=(k == 0), stop=(k == KT_HID - 1))
                            nc.scalar.copy(raw[:, c * T:(c + 1) * T], ps)
                        _rope2(nc, qt_sb[:, h, :], raw, cosq_sb, sinq_sb,
                               ropet)

                    qproj(0)
                    qproj(1)
                    for h in range(2, NH):
                        qproj(h)
                        attn_head(0, h - 2)
                        attn_head(1, h - 2)

                psD = pqa.enter_context(tc.tile_pool(
                    name="psD", bufs=2, space=bass.MemorySpace.PSUM))
                NCH = HID // T  # 2 chunks of 512

                def phase_d_tile(tt):
                    o_tile = opool.tile([P, HID], BF16, tag="o")
                    for cchunk in range(NCH):
                        ps = psD.tile([P, T], FP32, tag="dps")
                        for k in range(KT_HID):
                            nc.tensor.matmul(
                                ps,
                                ot_sb[:, k, tt * P:(tt + 1) * P],
                                wo_sb[:, k, cchunk * T:(cchunk + 1) * T],
                                start=(k == 0), stop=(k == KT_HID - 1))
                        # alternate engines so the copies run in parallel
                        if cchunk == 0:
                            nc.vector.tensor_copy(o_tile[:, 0:T], ps)
                        else:
                            nc.scalar.copy(o_tile[:, T:HID], ps)
                    nc.sync.dma_start(
                        out=out[tt * P:(tt + 1) * P, :], in_=o_tile)

                # flush the last heads; batch 0 finishes first so its
                # PE-dense output projection interleaves with batch 1's
                # ACT-bound final attention heads
                attn_head(0, NH - 2)
                attn_head(0, NH - 1)
                attn_head(1, NH - 2)
                phase_d_tile(0)
                attn_head(1, NH - 1)
                phase_d_tile(1)
                for tt in range(2, NTOK_T):
                    phase_d_tile(tt)


_COMPILED = None


def _get_compiled():
    global _COMPILED
    if _COMPILED is None:
        nc = bacc.Bacc("TRN2", target_bir_lowering=False, debug=False)
        _build(nc)
        nc.compile()
        _COMPILED = nc
    return _COMPILED


def kernel(hidden_states, Wq, Wk, Wv, Wo, _trace=False, _trace_kwargs=None):
    hs = np.asarray(hidden_states, dtype=np.float32).astype(BF)
    wq = np.ascontiguousarray(np.asarray(Wq, dtype=np.float32).astype(BF))
    wk = np.ascontiguousarray(np.asarray(Wk, dtype=np.float32).astype(BF))
    wv = np.ascontiguousarray(np.asarray(Wv, dtype=np.float32).astype(BF))
    wo = np.ascontiguousarray(np.asarray(Wo, dtype=np.float32).astype(BF))
    consts = _host_consts()
    nc = _get_compiled()
    in_maps = []
    for c in range(NCORES):
        # ship X pre-transposed ([HID, TOK]) so the kernel's lhs/rhs layouts
        # need no on-chip transpose of X at all
        shard_t = np.ascontiguousarray(
            hs[BL * c: BL * (c + 1)].reshape(TOK, HID).T)
        in_maps.append({"hidden_t": shard_t, "Wq": wq, "Wk": wk, "Wv": wv,
                        "Wo": wo, **consts})
    res = run_bass_kernel_spmd(
        nc, in_maps, list(range(NCORES)), trace=_trace,
        **(_trace_kwargs or {}))
    outs = [r["out"].astype(np.float32).reshape(BL, T, HID)
            for r in res.results]
    full = np.concatenate(outs, axis=0)
    if _trace:
        return full, res
    return full
